# revision 1
# baseline (speedup 1.0000x reference)
# 2D DCT-II [4096,4096] on 8 NeuronCores — v3 "four-step FFT" (bf16).
#
# Per core: RowS1 (block-diag DFT64 pairs, Hermitian-packed into 64 real
# planes) -> A2A (redistributes c-plane groups; the inter-stage transpose is
# folded into the wire format) -> RowS2 (per-output lhsT with twiddle+expk
# absorbed, per-core weights) -> XBAR transpose (rows<->storage-cols) ->
# ColS1 (same DFT64) -> plane-pair gather -> ColS2 (twiddle-absorbed lhsT).
# All data paths bf16, PSUM accumulation fp32. Host does only permutations
# (Makhoul reorder, column storage order, final unscramble) + dtype casts.
import numpy as np
import ml_dtypes
import concourse.bacc as bacc
import concourse.tile as tile
import concourse.mybir as mybir
from concourse import bass_utils

M = N = 4096
NC = 8
CPC = N // NC   # 512 cols per core

BF = ml_dtypes.bfloat16

# ---------------- weight/permutation construction (host) ----------------
W64C = np.exp(-2j * np.pi * np.arange(64)[:, None] * np.arange(64)[None, :] / 64)


def makhoul_perm(n):
    p = np.empty(n, dtype=np.int64)
    half = n // 2
    p[:half] = 2 * np.arange(half)
    p[half:] = 2 * (n - 1 - np.arange(half, n)) + 1
    return p


ROWP = makhoul_perm(M)
COLP = makhoul_perm(N)
MQ = (np.arange(N) % 64) * 64 + np.arange(N) // 64   # storage pos of v-col m


def wire_slots(r):
    if r == 0:
        return [(0, 0), (32, 0), (1, 0), (1, 1), (2, 0), (2, 1), (3, 0), (3, 1)]
    return [(4 * r + j // 2, j % 2) for j in range(8)]


def out_chat(r, o):
    j, v = o // 2, o % 2
    if r == 0 and j == 0:
        return 0 if v == 0 else 32
    ct = 4 * r + j
    return ct if v == 0 else 64 - ct


def w1_matrix():
    """lhsT for RowS1: [128, 128] block-diag; cols = wire planes."""
    w = np.zeros((64, 64))
    for r in range(NC):
        for ell, (ct, im) in enumerate(wire_slots(r)):
            col = 8 * r + ell
            w[:, col] = W64C[:, ct].imag if im else W64C[:, ct].real
    full = np.zeros((128, 128))
    full[:64, :64] = w
    full[64:, 64:] = w
    return full


def wc1_matrix():
    """lhsT for ColS1: [128, 128] block-diag; cols = plane order pos."""
    w = np.zeros((64, 64))
    for ct in range(32):
        w[:, 2 * ct] = W64C[:, ct].real
        if ct == 0:
            w[:, 1] = W64C[:, 32].real
        else:
            w[:, 2 * ct + 1] = W64C[:, ct].imag
    full = np.zeros((128, 128))
    full[:64, :64] = w
    full[64:, 64:] = w
    return full


def wr2_weights(r):
    """[8, 128, 64] for core r."""
    b = np.arange(64)
    d = np.arange(64)
    out = np.zeros((8, 128, 64))
    for o in range(8):
        chat = out_chat(r, o)
        alpha = np.exp(-1j * np.pi * chat / (2 * M))
        beta = np.exp(-1j * np.pi * d / 128)
        g = (alpha * np.exp(-2j * np.pi * b[:, None] * chat / M)
             * W64C[b][:, d] * beta[None, :])
        j, v = o // 2, o % 2
        if r == 0 and j == 0:
            if v == 0:
                out[o, :64] = g.real
            else:
                out[o, 64:] = g.real
        else:
            out[o, :64] = g.real
            out[o, 64:] = -g.imag if v == 0 else g.imag
    return out


def wc2_weights():
    """[64, 128, 64], uniform across cores; bt_of_p row permutation baked in."""
    b = np.arange(64)
    d = np.arange(64)
    p = np.arange(64)
    bt = 2 * (p % 32) + p // 32
    out = np.zeros((64, 128, 64))
    for oc in range(64):
        conj = oc > 32
        alpha = np.exp(-1j * np.pi * oc / (2 * N))
        beta = np.exp(-1j * np.pi * d / 128)
        g = (alpha * np.exp(-2j * np.pi * b[:, None] * oc / N)
             * W64C[b][:, d] * beta[None, :])
        if oc == 0:
            out[oc, :64] = g.real[bt]
        elif oc == 32:
            out[oc, 64:] = g.real[bt]
        else:
            out[oc, :64] = g.real[bt]
            out[oc, 64:] = (g.imag if conj else -g.imag)[bt]
    return out


# ---------------- bass kernel ----------------
_BUILT = {}


def build_nc(repeat=1, stage=8):
    dt = mybir.dt
    bf = dt.bfloat16
    nc = bacc.Bacc("TRN2", target_bir_lowering=False, debug=False, num_devices=NC)

    xin = nc.dram_tensor("xin", [128, 32, CPC], bf, kind="ExternalInput")
    w1 = nc.dram_tensor("w1", [128, 128], bf, kind="ExternalInput")
    wr2 = nc.dram_tensor("wr2", [128, 8, 64], bf, kind="ExternalInput")
    wc1 = nc.dram_tensor("wc1", [128, 128], bf, kind="ExternalInput")
    wc2 = nc.dram_tensor("wc2", [128, 64, 64], bf, kind="ExternalInput")
    ident = nc.dram_tensor("ident", [128, 128], bf, kind="ExternalInput")
    y = nc.dram_tensor("y", [128, 32, CPC], bf, kind="ExternalOutput")

    copy_engines = None  # set inside

    with tile.TileContext(nc) as tc:
        def copy(i, dst, src):
            if i % 2:
                nc.scalar.copy(dst, src)
            else:
                nc.vector.tensor_copy(dst, src)

        dma_eng = [nc.sync, nc.scalar]

        def dma(i, out, in_):
            dma_eng[i % 2].dma_start(out=out, in_=in_)

        with tc.tile_pool(name="dram", bufs=1, space="DRAM") as dram:
            z1 = dram.tile([64, 64, CPC], bf)
            z2 = dram.tile([64, 64, CPC], bf)
            for _rep in range(repeat):
                with (
                    tc.tile_pool(name="wts", bufs=1) as wts,
                ):
                    w1s = wts.tile([128, 128], bf, tag="w1s")
                    wr2s = wts.tile([128, 8, 64], bf, tag="wr2s")
                    wc1s = wts.tile([128, 128], bf, tag="wc1s")
                    wc2s = wts.tile([128, 64, 64], bf, tag="wc2s")
                    ids = wts.tile([128, 128], bf, tag="ids")
                    nc.scalar.dma_start(out=w1s[:], in_=w1[:])

                    # ---------- P0/P1: load xin + RowS1 ----------
                    with tc.tile_pool(name="pA", bufs=1) as pA:
                        xs = pA.tile([128, 32, CPC], bf, tag="xs")
                        o1 = pA.tile([128, 32, CPC], bf, tag="o1")
                        for h in range(4):
                            dma(h, xs[:, 8 * h:8 * (h + 1), :],
                                xin[:, 8 * h:8 * (h + 1), :])
                        zv = z1[:].rearrange("pl (q t) c -> pl q t c", t=2)
                        with tc.tile_pool(name="psA", bufs=4,
                                          space="PSUM") as psA:
                          for qp in range(16):
                            ps = psA.tile([128, 2 * CPC], dt.float32, tag="ps1")
                            for t in range(2):
                                nc.tensor.matmul(ps[:, t * CPC:(t + 1) * CPC],
                                                 w1s[:], xs[:, 2 * qp + t, :],
                                                 start=True, stop=True)
                            copy(qp, o1[:, 2 * qp:2 * qp + 2, :].rearrange(
                                "p q c -> p (q c)"), ps[:])
                            # ---- P2 interleaved: write z1 per 4-qp block ----
                            if qp % 4 == 3:
                                qs = slice(2 * (qp - 3), 2 * qp + 2)
                                dma(0, zv[:, qs, 0, :], o1[:64, qs, :])
                                dma(1, zv[:, qs, 1, :], o1[64:, qs, :])
                        nc.sync.dma_start(out=ids[:], in_=ident[:])
                        nc.scalar.dma_start(out=wr2s[:], in_=wr2[:])
                        nc.sync.dma_start(out=wc1s[:], in_=wc1[:])
                        nc.scalar.dma_start(out=wc2s[:], in_=wc2[:])
                        if stage <= 1:
                            dma(0, y[:], o1[:])
                    if stage <= 1:
                        continue

                    # ---------- P3: A2A ----------
                    nc.gpsimd.collective_compute(
                        "AllToAll", mybir.AluOpType.bypass,
                        replica_groups=[list(range(NC))],
                        ins=[z1[:].opt()], outs=[z2[:].opt()])

                    if stage <= 2:
                        with tc.tile_pool(name="pT", bufs=1) as pT:
                            tt = pT.tile([64, 64, CPC], bf, tag="tt")
                            dma(0, tt[:], z2[:])
                            dma(0, y[:64], tt[:, :32, :])
                        continue

                    # ---------- P4/P5: X2 + RowS2 ----------
                    with tc.tile_pool(name="pB", bufs=1) as pB:
                        x2 = pB.tile([128, 4, N], bf, tag="x2")
                        o2 = pB.tile([128, 4, N], bf, tag="o2")
                        z2v = z2[:].rearrange("(r l) b c -> r l b c", r=8)
                        for s in range(NC):
                            for half in range(2):
                                src = z2v[s, half::2]          # [4 l, 64 b, 512]
                                src = src.rearrange("l b c -> b l c")
                                dma(s * 2 + half,
                                    x2[64 * half:64 * (half + 1), :,
                                       s * CPC:(s + 1) * CPC], src)
                        if stage <= 3:
                            dma(0, y[:, :4, :], x2[:, :, :CPC])
                            continue
                        with tc.tile_pool(name="psB", bufs=4,
                                          space="PSUM") as psB:
                          for op in range(4):
                            for chp in range(4):
                                ps = psB.tile([128, 1024], dt.float32, tag="ps2")
                                for t in range(2):
                                    ch = 2 * chp + t
                                    nc.tensor.matmul(
                                        ps[:64, t * 512:(t + 1) * 512],
                                        wr2s[:, 2 * op, :],
                                        x2[:, op, 512 * ch:512 * (ch + 1)],
                                        start=True, stop=True)
                                    nc.tensor.matmul(
                                        ps[64:, t * 512:(t + 1) * 512],
                                        wr2s[:, 2 * op + 1, :],
                                        x2[:, op, 512 * ch:512 * (ch + 1)],
                                        start=True, stop=True)
                                copy(op * 4 + chp,
                                     o2[:, op, 1024 * chp:1024 * (chp + 1)],
                                     ps[:])

                        if stage <= 4:
                            dma(0, y[:, :4, :], o2[:, :, :CPC])
                            continue

                        # ---------- P6: PE transpose rows<->cols ----------
                        with tc.tile_pool(name="pC", bufs=1) as pC:
                            x3 = pC.tile([128, 32, 512], bf, tag="x3")
                            o3 = pC.tile([128, 4, 4096], bf, tag="o3")
                            with tc.tile_pool(name="psT", bufs=4,
                                              space="PSUM") as psT:
                              for csp in range(16):
                                ps = psT.tile([128, 1024], bf, tag="pst")
                                for t in range(2):
                                    cs = 2 * csp + t
                                    for op in range(4):
                                        nc.tensor.transpose(
                                            ps[:, 512 * t + 128 * op:
                                               512 * t + 128 * (op + 1)],
                                            o2[:, op, 128 * cs:128 * (cs + 1)],
                                            ids[:])
                                copy(csp, x3[:, 2 * csp:2 * csp + 2, :].rearrange(
                                    "p c r -> p (c r)"), ps[:])
                            if stage <= 5:
                                dma(0, y[:], x3[:])
                                continue
                            # ---------- P7: ColS1 (transposed: rows on parts) --
                            # o3 [128 rows, 4 rc, 4096 f], f = pos*64+bit*32+cs
                            o3r = o3[:].rearrange(
                                "p rc (pos bit cs) -> p rc pos bit cs",
                                pos=64, bit=2)
                            with tc.tile_pool(name="psC", bufs=4,
                                              space="PSUM") as psC:
                              for rc in range(4):
                                for csg in range(8):
                                    ps = psC.tile([128, 512], dt.float32,
                                                  tag="ps3")
                                    for csl in range(4):
                                        cs = 4 * csg + csl
                                        nc.tensor.matmul(
                                            ps[:, 128 * csl:128 * (csl + 1)],
                                            x3[:, cs, 128 * rc:128 * (rc + 1)],
                                            wc1s[:], start=True, stop=True)
                                    dst = o3r[:, rc, :, :, 4 * csg:4 * (csg + 1)]
                                    copy(rc * 8 + csg,
                                         dst.rearrange(
                                             "p pos bit csl -> p csl bit pos"),
                                         ps[:])

                            if stage <= 6:
                                dma(0, y[:], o3[:, 0, :].rearrange(
                                    "p (a c) -> p a c", a=32)[:, :32, :])
                                continue

                            # ---------- P8/P9: gap3 (PE transpose) + ColS2 --
                            with tc.tile_pool(name="pD", bufs=1) as pD:
                                x4 = pD.tile([128, 32, 512], bf, tag="x4")
                                yb = pD.tile([128, 32, 512], bf, tag="yb")
                                with tc.tile_pool(name="psU", bufs=4,
                                                  space="PSUM") as psU:
                                  for jp in range(16):
                                    ps = psU.tile([128, 1024], bf,
                                                  tag="psu")
                                    for t in range(2):
                                        j = 2 * jp + t
                                        for rc in range(4):
                                            nc.tensor.transpose(
                                                ps[:, 512 * t + 128 * rc:
                                                   512 * t + 128 * (rc + 1)],
                                                o3[:, rc, 128 * j:128 * (j + 1)],
                                                ids[:])
                                    copy(jp, x4[:, 2 * jp:2 * jp + 2, :]
                                         .rearrange("p c r -> p (c r)"), ps[:])
                                if stage <= 7:
                                    dma(0, y[:], x4[:])
                                    continue
                                with tc.tile_pool(name="psD", bufs=4,
                                                  space="PSUM") as psD:
                                  for opp in range(16):
                                    ps = psD.tile([128, 1024], dt.float32, tag="ps4")
                                    for t in range(2):
                                      for par in range(2):
                                        oc = 2 * (2 * opp + t) + par
                                        ct = oc if oc <= 32 else 64 - oc
                                        nc.tensor.matmul(
                                            ps[64 * par:64 * (par + 1),
                                               t * 512:(t + 1) * 512],
                                            wc2s[:, oc, :], x4[:, ct % 32, :],
                                            start=True, stop=True)
                                    copy(opp, yb[:, 2 * opp:2 * opp + 2, :].rearrange(
                                        "p o r -> p (o r)"), ps[:])
                                    if opp % 4 == 3:
                                        h = opp // 4
                                        dma(h, y[:, 8 * h:8 * (h + 1), :],
                                            yb[:, 8 * h:8 * (h + 1), :])

    nc.compile()
    return nc


# ---------------- host-side prep / unscramble ----------------
def _prep_weights():
    if "w1" in _BUILT:
        return
    _BUILT["w1"] = np.ascontiguousarray(w1_matrix().astype(BF))
    _BUILT["wc1"] = np.ascontiguousarray(wc1_matrix().astype(BF))
    wc2 = wc2_weights()                       # [64, 128, 64]
    _BUILT["wc2"] = np.ascontiguousarray(wc2.transpose(1, 0, 2).astype(BF))
    _BUILT["ident"] = np.ascontiguousarray(np.eye(128).astype(BF))
    _BUILT["wr2"] = [
        np.ascontiguousarray(wr2_weights(r).transpose(1, 0, 2).astype(BF))
        for r in range(NC)
    ]


def make_in_maps(x):
    """x float32 [4096, 4096] -> per-core input dicts."""
    _prep_weights()
    x = np.asarray(x, dtype=np.float32)
    vr = x[ROWP, :]
    xs = np.empty_like(vr)
    xs[:, MQ] = vr[:, COLP]
    xs = xs.astype(BF)
    maps = []
    for c in range(NC):
        strip = xs[:, c * CPC:(c + 1) * CPC]       # [4096 rows(v), 512]
        # xin[p, q, col]: p<64: vr[64p + 2q]; p>=64: vr[64(p-64) + 2q + 1]
        a = strip.reshape(64, 64, CPC)             # [a, b, col]
        xin = np.empty((128, 32, CPC), dtype=BF)
        xin[:64] = a[:, 0::2, :]
        xin[64:] = a[:, 1::2, :]
        maps.append({
            "xin": np.ascontiguousarray(xin),
            "w1": _BUILT["w1"],
            "wr2": _BUILT["wr2"][c],
            "wc1": _BUILT["wc1"],
            "wc2": _BUILT["wc2"],
            "ident": _BUILT["ident"],
        })
    return maps


def assemble(results):
    """results: list of per-core dicts with 'y' [128, 32, 512] bf16."""
    yfull = np.empty((M, N), dtype=np.float32)
    # row slots of core r (pass-1 rows): rowslot = op*128 + par*64 + d
    # -> o = 2*op + par, k = out_chat(r, o) + 64 d
    for r in range(NC):
        ks = np.empty(512, dtype=np.int64)
        for op in range(4):
            for par in range(2):
                o = 2 * op + par
                ks[op * 128 + par * 64:op * 128 + par * 64 + 64] = (
                    out_chat(r, o) + 64 * np.arange(64))
        yr = np.asarray(results[r]["y"]).astype(np.float32)  # [128, 32, 512]
        # y[p, ocp, row]: oc = 2*ocp + (p>=64), d' = p%64: k' = oc + 64 d'
        kps = np.empty((128, 32), dtype=np.int64)
        p = np.arange(128)
        for ocp in range(32):
            kps[:, ocp] = (2 * ocp + (p >= 64)) + 64 * (p % 64)
        # yfull[ks[row], kps[p, ocp]] = yr[p, ocp, row]
        flat = yr.transpose(2, 0, 1).reshape(512, 4096)   # [row, (p, ocp)]
        yfull[np.ix_(ks, kps.reshape(-1))] = flat
    return yfull


def kernel(x, expkM=None, expkN=None, trace=False):
    if "nc" not in _BUILT:
        _BUILT["nc"] = build_nc()
    nc = _BUILT["nc"]
    in_maps = make_in_maps(x)
    res = bass_utils.run_bass_kernel_spmd(nc, in_maps, core_ids=list(range(NC)),
                                          trace=trace)
    _BUILT["last_res"] = res
    return assemble(res.results).astype(np.float32)

